# revision 20
# baseline (speedup 1.0000x reference)
"""Trainium2 Bass kernel for the Agent forward pass (3 MLPs + KDE mixture).

Device computes layers 0-2 of the three MLPs (encoder / policy / MDN) in
feature-major layout (fp16 matmul operands, fp32 psum) and ships the final
hidden activations; host does the three tiny layer-3 projections, the KDE
tail (25 components x 3 dims per row), and the global-gradient-norm mix,
which needs a cross-shard reduction anyway.

Self-contained: hardcodes all shapes; imports only numpy + concourse.
"""

import os

import numpy as np

import concourse.bacc as bacc
import concourse.mybir as mybir
import concourse.tile as tile
from concourse.bass_utils import run_bass_kernel_spmd

# Problem dims (hardcoded per spec)
B = 131072
NCORES = 8
BC = B // NCORES  # 16384 rows per core
NG, ADIM = 25, 3
H = 1.0
NI = 0.0005
KDE_C = float((2.0 * np.pi * H**ADIM) ** (-0.5))

NB = 1024  # batch columns per slot
NSLOTS = BC // NB
NMM = 512  # matmul moving-operand chunk
G = 2  # slots per group: same-weight matmuls issue back-to-back

ACT_DT = mybir.dt.float16
ACT_NP = np.float16

# Engine for each relu: "A" = scalar/ACT, "V" = vector/DVE
ENG = {
    "r0e": "A", "r0m": "V", "r0p": "A",
    "r1e": "V", "r1m": "A", "r1p": "A",
    "r2e": "V", "r2m": "V", "r2p": "A",
}

# --- const pack column layout ---
_col = 0


def _take(n):
    global _col
    c = _col
    _col += n
    return c, _col


C_EW1 = _take(128)
C_EW2 = _take(128)
C_PW1 = _take(128)
C_PW2 = _take(128)
C_MW1 = _take(128)
C_MW2 = _take(128)
C_EW0 = _take(128)  # aug: rows 0-63 ew0, row 64 eb0
C_PW0 = _take(128)  # aug: rows 0-63 pw0[:64], row 64 pb0, rows 65-96 pw0[64:]
C_MW0 = _take(128)  # aug: row 64 mb0, rows 65-96 mw0
NCONST = _col

# f32 bias pack (per-partition bias vectors for relu ops)
B_EB1, B_MB1, B_PB1, B_EB2, B_MB2, B_PB2 = range(6)
NBIAS = 6


def _pack_consts(w):
    P = np.zeros((128, NCONST), ACT_NP)

    def put(cr, arr, r0=0):
        c0, c1 = cr
        a = np.asarray(arr, np.float32).astype(ACT_NP)
        P[r0 : r0 + a.shape[0], c0 : c0 + a.shape[1]] = a

    put(C_EW1, w["ew1"])
    put(C_EW2, w["ew2"])
    put(C_PW1, w["pw1"])
    put(C_PW2, w["pw2"])
    put(C_MW1, w["mw1"])
    put(C_MW2, w["mw2"])
    put(C_EW0, w["ew0"])
    put(C_EW0, w["eb0"][None, :], r0=64)
    put(C_PW0, w["pw0"][0:64])
    put(C_PW0, w["pb0"][None, :], r0=64)
    put(C_PW0, w["pw0"][64:96], r0=65)
    put(C_MW0, w["mb0"][None, :], r0=64)
    put(C_MW0, w["mw0"], r0=65)
    return P


def _pack_biases(w):
    Q = np.zeros((128, NBIAS), np.float32)
    for col, key in [(B_EB1, "eb1"), (B_MB1, "mb1"), (B_PB1, "pb1"),
                     (B_EB2, "eb2"), (B_MB2, "mb2"), (B_PB2, "pb2")]:
        Q[:, col] = np.asarray(w[key], np.float32)
    return Q


def build_program():
    """Build the per-core Bass program (same SPMD program on all 8 cores)."""
    nc = bacc.Bacc("TRN2", target_bir_lowering=False, debug=False)

    sg = nc.dram_tensor("sg", [128, BC], ACT_DT, kind="ExternalInput")
    wpack = nc.dram_tensor("wpack", [128, NCONST], ACT_DT, kind="ExternalInput")
    bpack = nc.dram_tensor("bpack", [128, NBIAS], mybir.dt.float32, kind="ExternalInput")
    out_e = nc.dram_tensor("out_e", [128, BC], ACT_DT, kind="ExternalOutput")
    out_m = nc.dram_tensor("out_m", [128, BC], ACT_DT, kind="ExternalOutput")
    out_p = nc.dram_tensor("out_p", [128, BC], ACT_DT, kind="ExternalOutput")

    relu = mybir.ActivationFunctionType.Relu
    add_op = mybir.AluOpType.add
    max_op = mybir.AluOpType.max

    with tile.TileContext(nc) as tc:
        with (
            tc.tile_pool(name="consts", bufs=1) as consts,
            tc.tile_pool(name="ins", bufs=6) as ins,
            tc.tile_pool(name="acts", bufs=8) as acts,
            tc.tile_pool(name="outs", bufs=6) as outs,
            tc.tile_pool(name="ps", bufs=4, space="PSUM") as ps,
        ):
            W = consts.tile([128, NCONST], ACT_DT)
            nc.sync.dma_start(out=W[:], in_=wpack[:])
            BV = consts.tile([128, NBIAS], mybir.dt.float32)
            nc.sync.dma_start(out=BV[:], in_=bpack[:])

            def wv(cr, r0=0, r1=128):
                c0, c1 = cr
                return W[r0:r1, c0:c1]

            def mm(out, lhsT, rhs):
                n = rhs.shape[-1]
                for j in range(0, n, NMM):
                    nc.tensor.matmul(
                        out[:, j : j + NMM], lhsT, rhs[:, j : j + NMM],
                        start=True, stop=True,
                    )

            def relu_bias(key, out, in_, bcol, eng=None):
                b = BV[:, bcol : bcol + 1]
                if (eng or ENG[key]) == "A":
                    nc.scalar.activation(out=out, in_=in_, func=relu, bias=b)
                else:
                    nc.vector.tensor_scalar(
                        out=out, in0=in_, scalar1=b, scalar2=0.0,
                        op0=add_op, op1=max_op,
                    )

            def relu_imm(key, out, in_):
                if ENG[key] == "A":
                    nc.scalar.activation(out=out, in_=in_, func=relu)
                else:
                    nc.vector.tensor_scalar_max(out=out, in0=in_, scalar1=0.0)

            _pn = [0]

            def psum():
                _pn[0] += 1
                return ps.tile(
                    [128, NB], mybir.dt.float32, tag="pnet", name=f"pp{_pn[0]}"
                )

            outd = {"e": out_e, "m": out_m, "p": out_p}

            for tp in range(0, NSLOTS, G):
                sgts, a1s, a2s = {}, {}, {}
                for t in range(tp, tp + G):
                    sgt = ins.tile([128, NB], ACT_DT, tag="sgt", name=f"sgt{t}")
                    nc.sync.dma_start(
                        out=sgt[:], in_=sg[:, t * NB : (t + 1) * NB]
                    )
                    sgts[t] = sgt

                # ---- layer 0 (biases folded via ones-row augmentation) ----
                for net, cr, r0, r1 in [
                    ("e", C_EW0, 0, 65), ("m", C_MW0, 64, 97), ("p", C_PW0, 0, 97)
                ]:
                    pps = {}
                    for t in range(tp, tp + G):
                        pp = psum()
                        mm(pp, wv(cr, r0, r1), sgts[t][r0:r1])
                        pps[t] = pp
                    for t in range(tp, tp + G):
                        a1 = acts.tile(
                            [128, NB], ACT_DT, tag=f"a1{net}", name=f"a1{net}{t}"
                        )
                        relu_imm(f"r0{net}", a1[:], pps[t][:])
                        a1s[(net, t)] = a1

                # ---- layer 1 ----
                for net, wcol, bcol in [
                    ("m", C_MW1, B_MB1), ("e", C_EW1, B_EB1), ("p", C_PW1, B_PB1)
                ]:
                    pps = {}
                    for t in range(tp, tp + G):
                        pp = psum()
                        mm(pp, wv(wcol), a1s[(net, t)][:])
                        pps[t] = pp
                    for t in range(tp, tp + G):
                        a2 = acts.tile(
                            [128, NB], ACT_DT, tag=f"a2{net}", name=f"a2{net}{t}"
                        )
                        # rebalance: ~1/5 of r1m instances run on DVE
                        ov = "V" if (net == "m" and t % 5 == 2) else None
                        relu_bias(f"r1{net}", a2[:], pps[t][:], bcol, eng=ov)
                        a2s[(net, t)] = a2

                # ---- layer 2: relu into a [128, G*NB] out tile, one DMA per net ----
                for net, wcol, bcol in [
                    ("e", C_EW2, B_EB2), ("p", C_PW2, B_PB2), ("m", C_MW2, B_MB2)
                ]:
                    pps = {}
                    for t in range(tp, tp + G):
                        pp = psum()
                        mm(pp, wv(wcol), a2s[(net, t)][:])
                        pps[t] = pp
                    a3 = outs.tile(
                        [128, G * NB], ACT_DT, tag=f"a3{net}", name=f"a3{net}{tp}"
                    )
                    for t in range(tp, tp + G):
                        j = (t - tp) * NB
                        relu_bias(f"r2{net}", a3[:, j : j + NB], pps[t][:], bcol)
                    nc.sync.dma_start(
                        out=outd[net][:, tp * NB : (tp + G) * NB], in_=a3[:]
                    )

    nc.compile()
    return nc


_NC = None
LAST_RESULTS = None  # BassKernelResults from the most recent run (for test.py)


def _get_nc():
    global _NC
    if _NC is None:
        _NC = build_program()
    return _NC


def kernel(**inputs):
    global LAST_RESULTS
    w = {k: np.asarray(v, np.float32) for k, v in inputs.items()}
    s, g = w["s"], w["g"]

    wpack = _pack_consts(w)
    bpack = _pack_biases(w)
    in_maps = []
    for c in range(NCORES):
        r0 = c * BC
        sgT = np.zeros((128, BC), ACT_NP)
        sgT[0:64] = s[r0 : r0 + BC].T.astype(ACT_NP)
        sgT[64] = 1.0
        sgT[65:97] = g[r0 : r0 + BC].T.astype(ACT_NP)
        in_maps.append(
            {"sg": np.ascontiguousarray(sgT), "wpack": wpack, "bpack": bpack}
        )

    nc = _get_nc()
    res = run_bass_kernel_spmd(
        nc,
        in_maps,
        core_ids=list(range(NCORES)),
        trace=bool(int(os.environ.get("KERNEL_TRACE", "0"))),
    )
    LAST_RESULTS = res

    a3e = np.empty((B, 128), np.float32)
    a3m = np.empty((B, 128), np.float32)
    a3p = np.empty((B, 128), np.float32)
    for c in range(NCORES):
        r0 = c * BC
        a3e[r0 : r0 + BC] = res.results[c]["out_e"].T
        a3m[r0 : r0 + BC] = res.results[c]["out_m"].T
        a3p[r0 : r0 + BC] = res.results[c]["out_p"].T

    # ---- host layer-3 projections ----
    z = a3e @ w["ew3"] + w["eb3"]
    mu = a3m @ w["mw3"] + w["mb3"]
    ail = a3p @ w["pw3"] + w["pb3"]

    # ---- host KDE tail + global-norm mix ----
    diff = z[:, None, :] - mu.reshape(B, NG, ADIM)  # [B, 25, 3]
    delta = -0.5 * np.einsum("bnd,bnd->bn", diff, diff) / (H * H)
    p = KDE_C * np.exp(delta)  # [B, 25]
    rho = p.sum(axis=-1)  # [B]
    grad = -np.einsum("bn,bnd->bd", p, diff) / (H * H)
    grad = np.nan_to_num(grad, nan=0.0)
    gnorm = np.linalg.norm(grad)
    gradn = grad / gnorm * NI
    pm = np.tanh(rho * 0.002)[:, None]
    out = pm * ail + (1.0 - pm) * gradn
    return out.astype(np.float32)
